# revision 1
# baseline (speedup 1.0000x reference)
"""Trainium2 Bass kernel for nn_CLF_block (channel-attention block).

Reference computation (per batch item b, with x = concat([a,b], ch) in [256, N],
N = H*W = 16384):
    z  = w1 x + b1 1^T
    q  = w2 z + b2 1^T ;  k = w3 z + b3 1^T ;  v = w4 z + b4 1^T
    qk = q k^T ; attn = softmax(qk, -1) ; out = attn v

Algebraic restructuring (verified vs reference, max-rel ~1e-4):
    Gx = x x^T                [256,256]   (one pass over x)
    sx = x 1                  [256]
    u  = w1 sx ; s = u + N b1
    G  = w1 Gx w1^T + u b1^T + b1 u^T + N b1 b1^T        (= z z^T)
    qk = w2 G w3^T + (w2 s) b3^T + b2 (w3 s)^T + N b2 b3^T
    attn = softmax(qk)
    M  = attn w4 ; W = M w1 ; c0 = M b1 + attn b4
    out = W x + c0 1^T        (second pass over x)

So only two O(256*256*N) passes over x touch HBM-sized data; everything else is
256x256 algebra. HBM traffic per core = 16 MiB in + 16 MiB out (x stays in SBUF
between the passes) -> memory-bound.

Sharding: data-parallel over batch, one batch item per NeuronCore (B=8, 8 cores).
"""

import sys

if "/opt/trn_rl_repo" not in sys.path:
    sys.path.insert(0, "/opt/trn_rl_repo")

from contextlib import ExitStack

import numpy as np

import concourse.bass as bass
import concourse.mybir as mybir
import concourse.tile as tile
from concourse import bacc
from concourse.bass_utils import run_bass_kernel_spmd

F32 = mybir.dt.float32
F32R = mybir.dt.float32r
F16 = mybir.dt.float16
P = 128          # partitions / channel block
C = 256          # channels
NPIX = 128 * 128  # spatial positions per batch item
NPIECE = 16       # resident x pieces per input half
PIECE = NPIX // NPIECE   # 1024 cols per piece
NCHUNK = NPIX // P       # 128 gram chunks
OUTW = 2048       # output staging tile width
NT = 512          # matmul moving-operand width for pass 2


def _emit(nc, tc, ctx, d_in, d_out):
    """Emit the Tile program for one core (one batch item)."""
    wcat, ident = d_in["wcat"], d_in["ident"]
    xht_d, xlt_d, xr_d = d_in["xht"], d_in["xlt"], d_in["xr"]
    brows, bcols = d_in["brows"], d_in["bcols"]
    out_d = d_out["out"]

    const = ctx.enter_context(tc.tile_pool(name="const", bufs=1))
    xpool = ctx.enter_context(tc.tile_pool(name="xpool", bufs=1))

    # --- constants -------------------------------------------------------
    w_sb = []
    for k in range(2):
        wt = const.tile([P, 5 * C], F32, name=f"w_sb{k}", tag=f"w_sb{k}")
        nc.sync.dma_start(out=wt, in_=wcat[k * P:(k + 1) * P, :])
        w_sb.append(wt)
    w1t = [w_sb[k][:, 0 * C:1 * C] for k in range(2)]   # w1^T  [cin, o]
    w1r = [w_sb[k][:, 1 * C:2 * C] for k in range(2)]   # w1    [o, cin]
    w2t = [w_sb[k][:, 2 * C:3 * C] for k in range(2)]   # w2^T
    w3t = [w_sb[k][:, 3 * C:4 * C] for k in range(2)]   # w3^T
    w4r = [w_sb[k][:, 4 * C:5 * C] for k in range(2)]   # w4    [d', d]

    rows = []
    for r in range(5):
        rt = const.tile([1, C], F32, name=f"brow{r}", tag=f"brow{r}")
        nc.sync.dma_start(out=rt, in_=brows[r:r + 1, :])
        rows.append(rt)
    b1_row, nb1_row, b2_row, b3_row, nb3_row = rows

    bc_sb = []
    for k in range(2):
        bt = const.tile([P, 4], F32, name=f"bcol{k}", tag=f"bcol{k}")
        nc.sync.dma_start(out=bt, in_=bcols[k * P:(k + 1) * P, :])
        bc_sb.append(bt)
    b1_col = [bc_sb[k][:, 0:1] for k in range(2)]
    nb1_col = [bc_sb[k][:, 1:2] for k in range(2)]
    b4_col = [bc_sb[k][:, 2:3] for k in range(2)]

    ident_sb = const.tile([P, P], F32R, name="ident_sb", tag="ident_sb")
    nc.sync.dma_start(out=ident_sb, in_=ident[:, :])

    # --- resident f32r-rounded x for pass 2 (two channel halves) ---------
    xs = [[], []]
    for c in range(2):
        eng = nc.sync if c == 0 else nc.scalar
        for i in range(NPIECE):
            xt = xpool.tile([P, PIECE], F32R, name=f"x{c}_{i}", tag=f"x{c}_{i}")
            eng.dma_start(out=xt,
                          in_=xr_d[c * P:(c + 1) * P,
                                   i * PIECE:(i + 1) * PIECE])
            xs[c].append(xt)

    # --- pass 1: Gx = x x^T via host-side fp16 split + transpose ---------
    # Host supplies xht (= xh^T chunks, ones-augmented) and xlt (= xl^T
    # chunks). Gx = Xh Xh^T + C' + C'^T with C' = Xl Xh^T (error ~2^-22).
    # Column 256 of shh/c accumulates sxh/sxl (exact row sums).
    gx_sb = [
        const.tile([P, C + 1], F32, name=f"gx_sb{b}", tag=f"gx_sb{b}")
        for b in range(2)
    ]
    c_sb = [
        const.tile([P, C + 1], F32, name=f"c_sb{b}", tag=f"c_sb{b}")
        for b in range(2)
    ]
    CH_PP = PIECE // P  # gram chunks per piece
    with tc.tile_pool(name="gx_ps", bufs=1, space="PSUM") as gxp, \
         tc.tile_pool(name="xt_sb", bufs=3) as xtp:
        shh_ps = [
            gxp.tile([P, C + 1], F32, name=f"shh_ps{b}", tag=f"shh{b}")
            for b in range(2)
        ]
        c_ps = [
            gxp.tile([P, C + 1], F32, name=f"c_ps{b}", tag=f"cps{b}")
            for b in range(2)
        ]
        for i in range(NPIECE):
            xht_p = xtp.tile([P, CH_PP, C + 1], F16, name="xht_p", tag="xht_p")
            xlt_p = xtp.tile([P, CH_PP, C], F16, name="xlt_p", tag="xlt_p")
            nc.sync.dma_start(out=xht_p, in_=xht_d[i])
            nc.scalar.dma_start(out=xlt_p, in_=xlt_d[i])
            for g in range(CH_PP):
                ch = i * CH_PP + g
                for b in range(2):
                    bs = slice(b * P, (b + 1) * P)
                    nc.tensor.matmul(shh_ps[b], xht_p[:, g, bs],
                                     xht_p[:, g, :],
                                     start=(ch == 0),
                                     stop=(ch == NCHUNK - 1))
                    nc.tensor.matmul(c_ps[b], xlt_p[:, g, bs],
                                     xht_p[:, g, :],
                                     start=(ch == 0),
                                     stop=(ch == NCHUNK - 1))
        for b in range(2):
            nc.vector.tensor_copy(gx_sb[b], shh_ps[b])
            nc.vector.tensor_scalar_mul(c_sb[b], c_ps[b], 1.0 / 2048.0)

    # Gx += C' + C'^T ; col 256: sx = sxh + sxl
    with tc.tile_pool(name="fix_ps", bufs=2, space="PSUM") as fxp:
        for b in range(2):
            nc.vector.tensor_add(gx_sb[b], gx_sb[b], c_sb[b])
        for b in range(2):
            for jb in range(2):
                ctp = fxp.tile([P, P], F32, name="ctp", tag="ctp")
                nc.tensor.transpose(ctp, c_sb[jb][:, b * P:(b + 1) * P],
                                    ident_sb.bitcast(F32))  # C'^T block
                nc.vector.tensor_add(gx_sb[b][:, jb * P:(jb + 1) * P],
                                     gx_sb[b][:, jb * P:(jb + 1) * P], ctp)

    # Split the (large) diagonal out of Gx: products (Gx-D) w1 are ~100x
    # smaller, so the PE's per-product rounding no longer pollutes qk.
    # The diagonal term is applied exactly via per-partition multiplies.
    gxd = []
    for b in range(2):
        bs = slice(b * P, (b + 1) * P)
        dm = const.tile([P, P], F32, name=f"gxdm{b}", tag=f"gxdm{b}")
        nc.vector.tensor_mul(dm, gx_sb[b][:, bs], ident_sb.bitcast(F32))
        dcol = const.tile([P, 1], F32, name=f"gxd{b}", tag=f"gxd{b}")
        nc.vector.reduce_sum(out=dcol, in_=dm, axis=mybir.AxisListType.X)
        nc.vector.tensor_sub(gx_sb[b][:, bs], gx_sb[b][:, bs], dm)
        gxd.append(dcol)

    # --- tiny 256x256 algebra -------------------------------------------
    # All matrices in SBUF as two [128, *] row-blocks; vectors as [1, C] rows
    # or [128, 1] per-block columns.
    alg_sb = const  # persistent small tiles live in the const pool

    with tc.tile_pool(name="alg_ps", bufs=3, space="PSUM") as ap:
        # u_row = (w1 sx)^T : lhsT = sx col (gx col 256), rhs = w1t
        u_row = alg_sb.tile([1, C], F32, name="u_row", tag="u_row")
        u_ps = ap.tile([1, C], F32, name="u_ps", tag="alg")
        for k in range(2):
            nc.tensor.matmul(u_ps, gx_sb[k][:, C:C + 1],
                             w1t[k].bitcast(F32),
                             start=(k == 0), stop=(k == 1))
        nc.vector.tensor_copy(u_row, u_ps)

        # U = (w1 Gx)^T : U[c, o] ; lhsT = Gx[c' blk k, c blk b], rhs = w1t[k]
        u_sb = []
        for b in range(2):
            ups = ap.tile([P, C], F32, name="ups", tag="alg")
            for k in range(2):
                nc.tensor.matmul(ups, gx_sb[k][:, b * P:(b + 1) * P],
                                 w1t[k].bitcast(F32),
                                 start=(k == 0), stop=(k == 1))
            ud = alg_sb.tile([P, C], F32, name=f"u_d{b}", tag=f"u_d{b}")
            nc.vector.tensor_scalar_mul(ud, w1t[b], gxd[b])
            ut = alg_sb.tile([P, C], F32, name=f"u_sb{b}", tag=f"u_sb{b}")
            nc.vector.tensor_add(ut, ups, ud)
            u_sb.append(ut)

        # G = U^T w1^T (+ rank-1 bias terms); u as column in separate psum
        g_sb = []
        g_diag = []
        for b in range(2):
            gps = ap.tile([P, C], F32, name="gps", tag="alg")
            ucps = ap.tile([P, 1], F32, name="ucps", tag="algsmall", bufs=2)
            for k in range(2):
                nc.tensor.matmul(gps,
                                 u_sb[k][:, b * P:(b + 1) * P].bitcast(F32),
                                 w1t[k].bitcast(F32), start=(k == 0),
                                 stop=False)
                # u_col block b: lhsT = w1t[k][:, b-slice], rhs = sx col
                nc.tensor.matmul(ucps,
                                 w1t[k][:, b * P:(b + 1) * P].bitcast(F32),
                                 gx_sb[k][:, C:C + 1],
                                 start=(k == 0), stop=(k == 1))
            nc.tensor.matmul(gps, u_row[:, b * P:(b + 1) * P], b1_row,
                             start=False, stop=False)
            nc.tensor.matmul(gps, b1_row[:, b * P:(b + 1) * P], u_row,
                             start=False, stop=False)
            nc.tensor.matmul(gps, b1_row[:, b * P:(b + 1) * P],
                             nb1_row, start=False, stop=True)
            gt = alg_sb.tile([P, C + 1], F32, name=f"g_sb{b}", tag=f"g_sb{b}")
            nc.vector.tensor_copy(gt[:, 0:C], gps)
            nc.vector.tensor_copy(gt[:, C:C + 1], ucps)
            bs = slice(b * P, (b + 1) * P)
            gdm = alg_sb.tile([P, P], F32, name=f"gdm{b}", tag=f"gdm{b}")
            nc.vector.tensor_mul(gdm, gt[:, bs], ident_sb.bitcast(F32))
            gdc = alg_sb.tile([P, 1], F32, name=f"gd{b}", tag=f"gd{b}")
            nc.vector.reduce_sum(out=gdc, in_=gdm, axis=mybir.AxisListType.X)
            nc.vector.tensor_sub(gt[:, bs], gt[:, bs], gdm)
            g_sb.append(gt)
            g_diag.append(gdc)

        # s_col = u_col + N*b1 (per block)
        s_col = []
        for k in range(2):
            st = alg_sb.tile([P, 1], F32, name=f"s_col{k}", tag=f"s_col{k}")
            nc.vector.tensor_add(st, g_sb[k][:, C:C + 1], nb1_col[k].bitcast(F32))
            s_col.append(st)

        # w2s_row = (w2 s)^T, w3s_row = (w3 s)^T
        w2s_row = alg_sb.tile([1, C], F32, name="w2s_row", tag="w2s_row")
        w3s_row = alg_sb.tile([1, C], F32, name="w3s_row", tag="w3s_row")
        for dst, wt in ((w2s_row, w2t), (w3s_row, w3t)):
            vps = ap.tile([1, C], F32, name="vps", tag="alg")
            for k in range(2):
                nc.tensor.matmul(vps, s_col[k].bitcast(F32),
                                 wt[k].bitcast(F32),
                                 start=(k == 0), stop=(k == 1))
            nc.vector.tensor_copy(dst, vps)

        # U2 = (w2 G)^T
        u2_sb = []
        for b in range(2):
            u2ps = ap.tile([P, C], F32, name="u2ps", tag="alg")
            for k in range(2):
                nc.tensor.matmul(u2ps, g_sb[k][:, b * P:(b + 1) * P].bitcast(F32),
                                 w2t[k].bitcast(F32),
                                 start=(k == 0), stop=(k == 1))
            u2d = alg_sb.tile([P, C], F32, name=f"u2_d{b}", tag=f"u2_d{b}")
            nc.vector.tensor_scalar_mul(u2d, w2t[b], g_diag[b])
            u2t = alg_sb.tile([P, C], F32, name=f"u2_sb{b}", tag=f"u2_sb{b}")
            nc.vector.tensor_add(u2t, u2ps, u2d)
            u2_sb.append(u2t)

        # qk = U2^T w3^T + rank-1 terms ; then softmax rows
        attn_sb = []
        for b in range(2):
            qkps = ap.tile([P, C], F32, name="qkps", tag="alg")
            for k in range(2):
                nc.tensor.matmul(qkps,
                                 u2_sb[k][:, b * P:(b + 1) * P].bitcast(F32),
                                 w3t[k].bitcast(F32), start=(k == 0),
                                 stop=False)
            nc.tensor.matmul(qkps, w2s_row[:, b * P:(b + 1) * P], b3_row,
                             start=False, stop=False)
            nc.tensor.matmul(qkps, b2_row[:, b * P:(b + 1) * P], w3s_row,
                             start=False, stop=False)
            nc.tensor.matmul(qkps, b2_row[:, b * P:(b + 1) * P], nb3_row,
                             start=False, stop=True)

            negmax = alg_sb.tile([P, 1], F32, name=f"negmax{b}", tag=f"nm{b}")
            nc.vector.tensor_reduce(
                out=negmax, in_=qkps, op=mybir.AluOpType.max,
                axis=mybir.AxisListType.X, negate=True,
            )
            expq = alg_sb.tile([P, C], F32, name=f"expq{b}", tag=f"expq{b}")
            nc.scalar.activation(
                out=expq, in_=qkps, func=mybir.ActivationFunctionType.Exp,
                bias=negmax, scale=1.0,
            )
            denom = alg_sb.tile([P, 1], F32, name=f"denom{b}", tag=f"dn{b}")
            nc.vector.reduce_sum(out=denom, in_=expq,
                                 axis=mybir.AxisListType.X)
            rden = alg_sb.tile([P, 1], F32, name=f"rden{b}", tag=f"rd{b}")
            nc.vector.reciprocal(rden, denom)
            at = alg_sb.tile([P, C], F32, name=f"attn{b}", tag=f"attn{b}")
            nc.vector.tensor_scalar_mul(at, expq, rden)
            attn_sb.append(at)

        # attn^T (4 PE transposes)
        attnT_sb = [
            alg_sb.tile([P, C], F32, name=f"attnT{j}", tag=f"attnT{j}")
            for j in range(2)
        ]
        for b in range(2):
            for j in range(2):
                tps = ap.tile([P, P], F32, name="tps", tag="algtp", bufs=2)
                nc.tensor.transpose(tps,
                                    attn_sb[b][:, j * P:(j + 1) * P],
                                    ident_sb.bitcast(F32))
                nc.vector.tensor_copy(attnT_sb[j][:, b * P:(b + 1) * P], tps)

        # M^T = w4-as-lhsT @ attn^T
        mt_sb = []
        for b in range(2):
            mps = ap.tile([P, C], F32, name="mps", tag="alg")
            for k in range(2):
                nc.tensor.matmul(mps, w4r[k][:, b * P:(b + 1) * P],
                                 (attnT_sb[k]), start=(k == 0), stop=(k == 1))
            mt = alg_sb.tile([P, C], F32, name=f"mt_sb{b}", tag=f"mt_sb{b}")
            nc.vector.tensor_copy(mt, mps)
            mt_sb.append(mt)

        # W^T = w1-as-lhsT @ M^T
        wt_sb = []
        for b in range(2):
            wps = ap.tile([P, C], F32, name="wps", tag="alg")
            for k in range(2):
                nc.tensor.matmul(wps, w1r[k][:, b * P:(b + 1) * P], mt_sb[k],
                                 start=(k == 0), stop=(k == 1))
            wt_ = alg_sb.tile([P, C], F32R, name=f"wt_sb{b}", tag=f"wt_sb{b}")
            nc.vector.tensor_copy(wt_, wps)
            wt_sb.append(wt_)

        # c0_col = M b1 + attn b4 (per block)
        c0_col = []
        for b in range(2):
            cps = ap.tile([P, 1], F32, name="cps", tag="alg")
            for k in range(2):
                nc.tensor.matmul(cps, mt_sb[k][:, b * P:(b + 1) * P].bitcast(F32),
                                 b1_col[k].bitcast(F32), start=(k == 0),
                                 stop=False)
            for k in range(2):
                nc.tensor.matmul(cps,
                                 attnT_sb[k][:, b * P:(b + 1) * P].bitcast(F32),
                                 b4_col[k].bitcast(F32), start=False,
                                 stop=(k == 1))
            ct = alg_sb.tile([P, 1], F32, name=f"c0_col{b}", tag=f"c0_col{b}")
            nc.vector.tensor_copy(ct, cps)
            c0_col.append(ct)

    # --- pass 2: out = W x + c0 1^T -------------------------------------
    # rhs x slices are rounded to f32r on the fly (7.6e-6 perturbation).
    with tc.tile_pool(name="o_ps", bufs=4, space="PSUM") as ops, \
         tc.tile_pool(name="o_sb", bufs=3) as osb, \
         tc.tile_pool(name="xr_sb", bufs=2) as xrp:
        nsub = PIECE // NT  # psum tiles per staging tile
        for i in range(NPIECE):
            xr = []
            for k in range(2):
                xrt = xrp.tile([P, PIECE], F32R, name=f"xr{k}", tag=f"xr{k}")
                nc.vector.tensor_copy(xrt, xs[k][i])
                xr.append(xrt)
            for b in range(2):
                ot = osb.tile([P, PIECE], F32, name="ot", tag="ot")
                pst = [
                    ops.tile([P, NT], F32, name="pst", tag="pst")
                    for _ in range(nsub)
                ]
                for k in range(2):
                    for t in range(nsub):
                        nc.tensor.matmul(
                            pst[t],
                            wt_sb[k][:, b * P:(b + 1) * P],
                            xr[k][:, t * NT:(t + 1) * NT],
                            start=(k == 0),
                            stop=(k == 1),
                        )
                for t in range(nsub):
                    nc.scalar.activation(
                        out=ot[:, t * NT:(t + 1) * NT], in_=pst[t],
                        func=mybir.ActivationFunctionType.Identity,
                        bias=c0_col[b], scale=1.0,
                    )
                (nc.sync if b == 0 else nc.scalar).dma_start(
                    out=out_d[b * P:(b + 1) * P, i * PIECE:(i + 1) * PIECE],
                    in_=ot,
                )


def build_program(enable_asserts=False):
    nc = bacc.Bacc(
        "TRN2",
        target_bir_lowering=False,
        debug=False,
        enable_asserts=enable_asserts,
        num_devices=8,
    )
    d_in = {
        "xht": nc.dram_tensor("xht", [NPIECE, P, PIECE // P, C + 1],
                              mybir.dt.float16, kind="ExternalInput").ap(),
        "xlt": nc.dram_tensor("xlt", [NPIECE, P, PIECE // P, C],
                              mybir.dt.float16, kind="ExternalInput").ap(),
        "xr": nc.dram_tensor("xr", [C, NPIX], F32R,
                             kind="ExternalInput").ap(),
        "wcat": nc.dram_tensor("wcat", [C, 5 * C], F32,
                               kind="ExternalInput").ap(),
        "brows": nc.dram_tensor("brows", [5, C], F32,
                                kind="ExternalInput").ap(),
        "bcols": nc.dram_tensor("bcols", [C, 4], F32,
                                kind="ExternalInput").ap(),
        "ident": nc.dram_tensor("ident", [P, P], F32R,
                                kind="ExternalInput").ap(),
    }
    d_out = {
        "out": nc.dram_tensor("out", [C, NPIX], F32,
                              kind="ExternalOutput").ap(),

    }
    with tile.TileContext(nc) as tc, ExitStack() as ctx:
        _emit(nc, tc, ctx, d_in, d_out)
    nc.compile()
    return nc


def _round_f32r(x):
    """Round fp32 to the FP32R-representable set (hi-bf16 + lo-bf16)."""
    import ml_dtypes

    x = np.asarray(x, np.float32)
    hi = x.astype(ml_dtypes.bfloat16).astype(np.float32)
    lo = (x - hi).astype(ml_dtypes.bfloat16).astype(np.float32)
    return hi + lo


def make_in_maps(a, b, w1, b1, w2, b2, w3, b3, w4, b4):
    N = NPIX
    f = np.float32
    wcat = np.concatenate([w1.T, w1, w2.T, w3.T, w4],
                          axis=1).astype(f, copy=False)
    brows = np.stack([b1, N * b1, b2, b3, N * b3]).astype(f, copy=False)
    bcols = np.stack([b1, N * b1, b4, np.ones(C, f)], axis=1).astype(f)
    ident = np.eye(P, dtype=f)
    B = a.shape[0]
    g = PIECE // P
    in_maps = []
    for i in range(B):
        x = np.concatenate([a[i].reshape(P, N), b[i].reshape(P, N)], axis=0)
        xh = x.astype(np.float16)
        # scale xl into fp16 normal range (PE flushes fp16 subnormals);
        # the kernel rescales the C' term by 1/2048.
        xl = ((x - xh.astype(f)) * 2048.0).astype(np.float16)
        xht = np.ascontiguousarray(
            xh.T.reshape(NPIECE, g, P, C).transpose(0, 2, 1, 3))
        ones = np.ones((NPIECE, P, g, 1), np.float16)
        xht = np.ascontiguousarray(np.concatenate([xht, ones], axis=3))
        xlt = np.ascontiguousarray(
            xl.T.reshape(NPIECE, g, P, C).transpose(0, 2, 1, 3))
        in_maps.append({
            "xht": xht,
            "xlt": xlt,
            "xr": _round_f32r(x),
            "wcat": wcat,
            "brows": brows,
            "bcols": bcols,
            "ident": ident,
        })
    return in_maps


_CACHE = {}


def kernel(a, b, w1, b1, w2, b2, w3, b3, w4, b4, _trace=False):
    a = np.asarray(a, dtype=np.float32)
    b = np.asarray(b, dtype=np.float32)
    args = [np.asarray(t, dtype=np.float32)
            for t in (w1, b1, w2, b2, w3, b3, w4, b4)]
    if "nc" not in _CACHE:
        _CACHE["nc"] = build_program()
    nc = _CACHE["nc"]
    in_maps = make_in_maps(a, b, *args)
    res = run_bass_kernel_spmd(nc, in_maps, core_ids=list(range(8)),
                               trace=_trace)
    B, Ch, H, W = a.shape
    out = np.stack([r["out"].reshape(C, H, W) for r in res.results])
    if _trace:
        _CACHE["last_results"] = res
    return out



# revision 4
# speedup vs baseline: 1.4591x; 1.4591x over previous
"""Trainium2 Bass kernel for nn_CLF_block (channel-attention block).

Reference computation (per batch item b, with x = concat([a,b], ch) in [256, N],
N = H*W = 16384):
    z  = w1 x + b1 1^T
    q  = w2 z + b2 1^T ;  k = w3 z + b3 1^T ;  v = w4 z + b4 1^T
    qk = q k^T ; attn = softmax(qk, -1) ; out = attn v

Algebraic restructuring (as in the original two-pass scheme):
    Gx = x x^T                [256,256]   (one pass over x)
    sx = x 1                  [256]
    u  = w1 sx ; s = u + N b1
    G  = w1 Gx w1^T + u b1^T + b1 u^T + N b1 b1^T        (= z z^T)
    qk = w2 G w3^T + (w2 s) b3^T + b2 (w3 s)^T + N b2 b3^T
    attn = softmax(qk)
    M  = attn w4 ; W = M w1 ; c0 = M b1 + attn b4
    out = W x + c0 1^T        (second pass over x)

This version minimizes HBM traffic: x is downloaded ONCE as fp16 in natural
layout (8 MiB/core).  The gram pass needs x^T chunks, which are produced
on-chip with PE transposes (fp16 transpose = 1 cycle/row); Gx symmetry is
exploited so block (1,0) is recovered as block (0,1)^T.  The fp16 lo-part
correction of the original kernel is dropped entirely (fp16-only gram gives
~3e-3 rel err vs the 2e-2 gate).  Pass 2 runs fp16 x fp16 and the output is
written as fp16 (8 MiB/core), upcast on the host.

HBM per core: ~9.3 MiB in + 8 MiB out (vs 48 MiB in the two-download scheme).

Sharding: data-parallel over batch, one batch item per NeuronCore (B=8, 8 cores).
"""

import sys

if "/opt/trn_rl_repo" not in sys.path:
    sys.path.insert(0, "/opt/trn_rl_repo")

from contextlib import ExitStack

import numpy as np

import concourse.bass as bass
import concourse.mybir as mybir
import concourse.tile as tile
from concourse import bacc
from concourse.bass_utils import run_bass_kernel_spmd

F32 = mybir.dt.float32
F32R = mybir.dt.float32r
F16 = mybir.dt.float16
P = 128           # partitions / channel block
C = 256           # channels
NPIX = 128 * 128  # spatial positions per batch item
NPIECE = 16       # x DMA pieces per channel half
PIECE = NPIX // NPIECE   # 1024 cols per piece
NCHUNK = NPIX // P       # 128 gram chunks
GRP = 8           # chunks per transpose group (one PSUM bank = 8 x [128,128] fp16)
NGRP = NCHUNK // GRP     # 16 groups
CH_PP = PIECE // P       # 8 chunks per piece
NT = 512          # matmul moving-operand width for pass 2


def _emit(nc, tc, ctx, d_in, d_out):
    """Emit the Tile program for one core (one batch item)."""
    wcat, ident, identh = d_in["wcat"], d_in["ident"], d_in["identh"]
    x0_d, x1_d = d_in["x0"], d_in["x1"]
    brows, bcols = d_in["brows"], d_in["bcols"]
    out_d = d_out["out"]

    const = ctx.enter_context(tc.tile_pool(name="const", bufs=1))
    xpool = ctx.enter_context(tc.tile_pool(name="xpool", bufs=1))

    # --- constants -------------------------------------------------------
    w_sb = []
    for k in range(2):
        wt = const.tile([P, 5 * C], F32, name=f"w_sb{k}", tag=f"w_sb{k}")
        nc.scalar.dma_start(out=wt, in_=wcat[k * P:(k + 1) * P, :])
        w_sb.append(wt)
    w1t = [w_sb[k][:, 0 * C:1 * C] for k in range(2)]   # w1^T  [cin, o]
    w1r = [w_sb[k][:, 1 * C:2 * C] for k in range(2)]   # w1    [o, cin]
    w2t = [w_sb[k][:, 2 * C:3 * C] for k in range(2)]   # w2^T
    w3t = [w_sb[k][:, 3 * C:4 * C] for k in range(2)]   # w3^T
    w4r = [w_sb[k][:, 4 * C:5 * C] for k in range(2)]   # w4    [d', d]

    rows = []
    for r in range(5):
        rt = const.tile([1, C], F32, name=f"brow{r}", tag=f"brow{r}")
        nc.scalar.dma_start(out=rt, in_=brows[r:r + 1, :])
        rows.append(rt)
    b1_row, nb1_row, b2_row, b3_row, nb3_row = rows

    bc_sb = []
    for k in range(2):
        bt = const.tile([P, 4], F32, name=f"bcol{k}", tag=f"bcol{k}")
        nc.scalar.dma_start(out=bt, in_=bcols[k * P:(k + 1) * P, :])
        bc_sb.append(bt)
    b1_col = [bc_sb[k][:, 0:1] for k in range(2)]
    nb1_col = [bc_sb[k][:, 1:2] for k in range(2)]
    b4_col = [bc_sb[k][:, 2:3] for k in range(2)]

    ident_sb = const.tile([P, P], F32R, name="ident_sb", tag="ident_sb")
    nc.scalar.dma_start(out=ident_sb, in_=ident[:, :])
    identh_sb = const.tile([P, P], F16, name="identh_sb", tag="identh_sb")
    nc.sync.dma_start(out=identh_sb, in_=identh[:, :])

    # --- resident fp16 x in natural layout (two channel halves) ----------
    xs = [[], []]
    for c, src in ((0, x0_d), (1, x1_d)):
        eng = nc.sync if c == 0 else nc.scalar
        for i in range(NPIECE):
            xt = xpool.tile([P, PIECE], F16, name=f"x{c}_{i}", tag=f"x{c}_{i}")
            eng.dma_start(out=xt, in_=src[:, i * PIECE:(i + 1) * PIECE])
            xs[c].append(xt)

    # --- pass 1: Gx = x x^T via on-chip PE transposes --------------------
    # Per 128-col chunk: transpose both channel-half chunks into PSUM, batch
    # 4 chunks per bank, copy to an ones-augmented fp16 staging tile, then
    # two accumulating gram matmuls.  Symmetry: only blocks (0,*) and (1,1)
    # are computed; block (1,0) = block (0,1)^T afterwards.
    stg = []
    for s in range(2):
        st = const.tile([P, GRP, C + 1], F16, name=f"stg{s}", tag=f"stg{s}")
        nc.vector.memset(st[:, :, C:C + 1], 1.0)
        stg.append(st)

    gx_sb = [
        const.tile([P, C + 1], F32, name=f"gx_sb{b}", tag=f"gx_sb{b}")
        for b in range(2)
    ]

    with tc.tile_pool(name="gx_ps", bufs=1, space="PSUM") as gxp, \
         tc.tile_pool(name="tp_ps", bufs=1, space="PSUM") as tpp:
        shh0 = gxp.tile([P, C + 1], F32, name="shh0", tag="shh0")
        shh1 = gxp.tile([P, P + 1], F32, name="shh1", tag="shh1")

        def emit_transposes(grp):
            s = grp % 2
            tpA = tpp.tile([P, GRP, P], F16, name="tpA", tag=f"tpA{s}")
            tpB = tpp.tile([P, GRP, P], F16, name="tpB", tag=f"tpB{s}")
            for g in range(GRP):
                ch = grp * GRP + g
                i, lc = divmod(ch, CH_PP)
                nc.tensor.transpose(tpA[:, g, :],
                                    xs[0][i][:, lc * P:(lc + 1) * P],
                                    identh_sb)
                nc.tensor.transpose(tpB[:, g, :],
                                    xs[1][i][:, lc * P:(lc + 1) * P],
                                    identh_sb)
            nc.vector.tensor_copy(stg[s][:, :, 0:P], tpA)
            nc.scalar.activation(out=stg[s][:, :, P:C], in_=tpB,
                                 func=mybir.ActivationFunctionType.Identity,
                                 scale=1.0)

        def emit_grams(grp):
            s = grp % 2
            for g in range(GRP):
                ch = grp * GRP + g
                nc.tensor.matmul(shh0, stg[s][:, g, 0:P],
                                 stg[s][:, g, 0:C + 1],
                                 start=(ch == 0), stop=(ch == NCHUNK - 1))
                nc.tensor.matmul(shh1, stg[s][:, g, P:C],
                                 stg[s][:, g, P:C + 1],
                                 start=(ch == 0), stop=(ch == NCHUNK - 1))

        for grp in range(NGRP + 1):
            if grp < NGRP:
                emit_transposes(grp)
            if grp >= 1:
                emit_grams(grp - 1)

        # Assemble full Gx (with sx in col 256) from the symmetric pieces.
        nc.vector.tensor_copy(gx_sb[0], shh0)
        nc.vector.tensor_copy(gx_sb[1][:, P:C + 1], shh1)
        with tc.tile_pool(name="sym_ps", bufs=1, space="PSUM") as syp:
            tps = syp.tile([P, P], F32, name="tps", tag="tps")
            nc.tensor.transpose(tps, gx_sb[0][:, P:C],
                                ident_sb.bitcast(F32))
            nc.vector.tensor_copy(gx_sb[1][:, 0:P], tps)

    # Split the (large) diagonal out of Gx: products (Gx-D) w1 are ~100x
    # smaller, so the PE's per-product rounding no longer pollutes qk.
    # The diagonal term is applied exactly via per-partition multiplies.
    gxd = []
    for b in range(2):
        bs = slice(b * P, (b + 1) * P)
        dm = const.tile([P, P], F32, name=f"gxdm{b}", tag=f"gxdm{b}")
        nc.vector.tensor_mul(dm, gx_sb[b][:, bs], ident_sb.bitcast(F32))
        dcol = const.tile([P, 1], F32, name=f"gxd{b}", tag=f"gxd{b}")
        nc.vector.reduce_sum(out=dcol, in_=dm, axis=mybir.AxisListType.X)
        nc.vector.tensor_sub(gx_sb[b][:, bs], gx_sb[b][:, bs], dm)
        gxd.append(dcol)

    # --- tiny 256x256 algebra -------------------------------------------
    # All matrices in SBUF as two [128, *] row-blocks; vectors as [1, C] rows
    # or [128, 1] per-block columns.
    alg_sb = const  # persistent small tiles live in the const pool

    with tc.tile_pool(name="alg_ps", bufs=3, space="PSUM") as ap:
        # u_row = (w1 sx)^T : lhsT = sx col (gx col 256), rhs = w1t
        u_row = alg_sb.tile([1, C], F32, name="u_row", tag="u_row")
        u_ps = ap.tile([1, C], F32, name="u_ps", tag="alg")
        for k in range(2):
            nc.tensor.matmul(u_ps, gx_sb[k][:, C:C + 1],
                             w1t[k].bitcast(F32),
                             start=(k == 0), stop=(k == 1))
        nc.vector.tensor_copy(u_row, u_ps)

        # U = (w1 Gx)^T : U[c, o] ; lhsT = Gx[c' blk k, c blk b], rhs = w1t[k]
        u_sb = []
        for b in range(2):
            ups = ap.tile([P, C], F32, name="ups", tag="alg")
            for k in range(2):
                nc.tensor.matmul(ups, gx_sb[k][:, b * P:(b + 1) * P],
                                 w1t[k].bitcast(F32),
                                 start=(k == 0), stop=(k == 1))
            ud = alg_sb.tile([P, C], F32, name=f"u_d{b}", tag=f"u_d{b}")
            nc.vector.tensor_scalar_mul(ud, w1t[b], gxd[b])
            ut = alg_sb.tile([P, C], F32, name=f"u_sb{b}", tag=f"u_sb{b}")
            nc.vector.tensor_add(ut, ups, ud)
            u_sb.append(ut)

        # G = U^T w1^T (+ rank-1 bias terms); u as column in separate psum
        g_sb = []
        g_diag = []
        for b in range(2):
            gps = ap.tile([P, C], F32, name="gps", tag="alg")
            ucps = ap.tile([P, 1], F32, name="ucps", tag="algsmall", bufs=2)
            for k in range(2):
                nc.tensor.matmul(gps,
                                 u_sb[k][:, b * P:(b + 1) * P].bitcast(F32),
                                 w1t[k].bitcast(F32), start=(k == 0),
                                 stop=False)
                # u_col block b: lhsT = w1t[k][:, b-slice], rhs = sx col
                nc.tensor.matmul(ucps,
                                 w1t[k][:, b * P:(b + 1) * P].bitcast(F32),
                                 gx_sb[k][:, C:C + 1],
                                 start=(k == 0), stop=(k == 1))
            nc.tensor.matmul(gps, u_row[:, b * P:(b + 1) * P], b1_row,
                             start=False, stop=False)
            nc.tensor.matmul(gps, b1_row[:, b * P:(b + 1) * P], u_row,
                             start=False, stop=False)
            nc.tensor.matmul(gps, b1_row[:, b * P:(b + 1) * P],
                             nb1_row, start=False, stop=True)
            gt = alg_sb.tile([P, C + 1], F32, name=f"g_sb{b}", tag=f"g_sb{b}")
            nc.vector.tensor_copy(gt[:, 0:C], gps)
            nc.vector.tensor_copy(gt[:, C:C + 1], ucps)
            bs = slice(b * P, (b + 1) * P)
            gdm = alg_sb.tile([P, P], F32, name=f"gdm{b}", tag=f"gdm{b}")
            nc.vector.tensor_mul(gdm, gt[:, bs], ident_sb.bitcast(F32))
            gdc = alg_sb.tile([P, 1], F32, name=f"gd{b}", tag=f"gd{b}")
            nc.vector.reduce_sum(out=gdc, in_=gdm, axis=mybir.AxisListType.X)
            nc.vector.tensor_sub(gt[:, bs], gt[:, bs], gdm)
            g_sb.append(gt)
            g_diag.append(gdc)

        # s_col = u_col + N*b1 (per block)
        s_col = []
        for k in range(2):
            st = alg_sb.tile([P, 1], F32, name=f"s_col{k}", tag=f"s_col{k}")
            nc.vector.tensor_add(st, g_sb[k][:, C:C + 1], nb1_col[k].bitcast(F32))
            s_col.append(st)

        # w2s_row = (w2 s)^T, w3s_row = (w3 s)^T
        w2s_row = alg_sb.tile([1, C], F32, name="w2s_row", tag="w2s_row")
        w3s_row = alg_sb.tile([1, C], F32, name="w3s_row", tag="w3s_row")
        for dst, wt in ((w2s_row, w2t), (w3s_row, w3t)):
            vps = ap.tile([1, C], F32, name="vps", tag="alg")
            for k in range(2):
                nc.tensor.matmul(vps, s_col[k].bitcast(F32),
                                 wt[k].bitcast(F32),
                                 start=(k == 0), stop=(k == 1))
            nc.vector.tensor_copy(dst, vps)

        # U2 = (w2 G)^T
        u2_sb = []
        for b in range(2):
            u2ps = ap.tile([P, C], F32, name="u2ps", tag="alg")
            for k in range(2):
                nc.tensor.matmul(u2ps, g_sb[k][:, b * P:(b + 1) * P].bitcast(F32),
                                 w2t[k].bitcast(F32),
                                 start=(k == 0), stop=(k == 1))
            u2d = alg_sb.tile([P, C], F32, name=f"u2_d{b}", tag=f"u2_d{b}")
            nc.vector.tensor_scalar_mul(u2d, w2t[b], g_diag[b])
            u2t = alg_sb.tile([P, C], F32, name=f"u2_sb{b}", tag=f"u2_sb{b}")
            nc.vector.tensor_add(u2t, u2ps, u2d)
            u2_sb.append(u2t)

        # qk = U2^T w3^T + rank-1 terms ; then softmax rows
        attn_sb = []
        for b in range(2):
            qkps = ap.tile([P, C], F32, name="qkps", tag="alg")
            for k in range(2):
                nc.tensor.matmul(qkps,
                                 u2_sb[k][:, b * P:(b + 1) * P].bitcast(F32),
                                 w3t[k].bitcast(F32), start=(k == 0),
                                 stop=False)
            nc.tensor.matmul(qkps, w2s_row[:, b * P:(b + 1) * P], b3_row,
                             start=False, stop=False)
            nc.tensor.matmul(qkps, b2_row[:, b * P:(b + 1) * P], w3s_row,
                             start=False, stop=False)
            nc.tensor.matmul(qkps, b2_row[:, b * P:(b + 1) * P],
                             nb3_row, start=False, stop=True)

            negmax = alg_sb.tile([P, 1], F32, name=f"negmax{b}", tag=f"nm{b}")
            nc.vector.tensor_reduce(
                out=negmax, in_=qkps, op=mybir.AluOpType.max,
                axis=mybir.AxisListType.X, negate=True,
            )
            expq = alg_sb.tile([P, C], F32, name=f"expq{b}", tag=f"expq{b}")
            nc.scalar.activation(
                out=expq, in_=qkps, func=mybir.ActivationFunctionType.Exp,
                bias=negmax, scale=1.0,
            )
            denom = alg_sb.tile([P, 1], F32, name=f"denom{b}", tag=f"dn{b}")
            nc.vector.reduce_sum(out=denom, in_=expq,
                                 axis=mybir.AxisListType.X)
            rden = alg_sb.tile([P, 1], F32, name=f"rden{b}", tag=f"rd{b}")
            nc.vector.reciprocal(rden, denom)
            at = alg_sb.tile([P, C], F32, name=f"attn{b}", tag=f"attn{b}")
            nc.vector.tensor_scalar_mul(at, expq, rden)
            attn_sb.append(at)

        # attn^T (4 PE transposes)
        attnT_sb = [
            alg_sb.tile([P, C], F32, name=f"attnT{j}", tag=f"attnT{j}")
            for j in range(2)
        ]
        for b in range(2):
            for j in range(2):
                tps = ap.tile([P, P], F32, name="tps", tag="algtp", bufs=2)
                nc.tensor.transpose(tps,
                                    attn_sb[b][:, j * P:(j + 1) * P],
                                    ident_sb.bitcast(F32))
                nc.vector.tensor_copy(attnT_sb[j][:, b * P:(b + 1) * P], tps)

        # M^T = w4-as-lhsT @ attn^T
        mt_sb = []
        for b in range(2):
            mps = ap.tile([P, C], F32, name="mps", tag="alg")
            for k in range(2):
                nc.tensor.matmul(mps, w4r[k][:, b * P:(b + 1) * P],
                                 (attnT_sb[k]), start=(k == 0), stop=(k == 1))
            mt = alg_sb.tile([P, C], F32, name=f"mt_sb{b}", tag=f"mt_sb{b}")
            nc.vector.tensor_copy(mt, mps)
            mt_sb.append(mt)

        # W^T = w1-as-lhsT @ M^T  (stored fp16 for the fp16 pass 2)
        wt_sb = []
        for b in range(2):
            wps = ap.tile([P, C], F32, name="wps", tag="alg")
            for k in range(2):
                nc.tensor.matmul(wps, w1r[k][:, b * P:(b + 1) * P], mt_sb[k],
                                 start=(k == 0), stop=(k == 1))
            wt_ = alg_sb.tile([P, C], F16, name=f"wt_sb{b}", tag=f"wt_sb{b}")
            nc.vector.tensor_copy(wt_, wps)
            wt_sb.append(wt_)

        # c0_col = M b1 + attn b4 (per block)
        c0_col = []
        for b in range(2):
            cps = ap.tile([P, 1], F32, name="cps", tag="alg")
            for k in range(2):
                nc.tensor.matmul(cps, mt_sb[k][:, b * P:(b + 1) * P].bitcast(F32),
                                 b1_col[k].bitcast(F32), start=(k == 0),
                                 stop=False)
            for k in range(2):
                nc.tensor.matmul(cps,
                                 attnT_sb[k][:, b * P:(b + 1) * P].bitcast(F32),
                                 b4_col[k].bitcast(F32), start=False,
                                 stop=(k == 1))
            ct = alg_sb.tile([P, 1], F32, name=f"c0_col{b}", tag=f"c0_col{b}")
            nc.vector.tensor_copy(ct, cps)
            c0_col.append(ct)

    # --- pass 2: out = W x + c0 1^T (all fp16 operands, fp16 output) -----
    with tc.tile_pool(name="o_ps", bufs=4, space="PSUM") as ops, \
         tc.tile_pool(name="o_sb", bufs=3) as osb:
        nsub = PIECE // NT  # psum tiles per staging tile
        for i in range(NPIECE):
            for b in range(2):
                ot = osb.tile([P, PIECE], F16, name="ot", tag="ot")
                pst = [
                    ops.tile([P, NT], F32, name="pst", tag="pst")
                    for _ in range(nsub)
                ]
                for k in range(2):
                    for t in range(nsub):
                        nc.tensor.matmul(
                            pst[t],
                            wt_sb[k][:, b * P:(b + 1) * P],
                            xs[k][i][:, t * NT:(t + 1) * NT],
                            start=(k == 0),
                            stop=(k == 1),
                        )
                for t in range(nsub):
                    if b == 0:
                        nc.scalar.activation(
                            out=ot[:, t * NT:(t + 1) * NT], in_=pst[t],
                            func=mybir.ActivationFunctionType.Identity,
                            bias=c0_col[b], scale=1.0,
                        )
                    else:
                        nc.vector.tensor_scalar_add(
                            ot[:, t * NT:(t + 1) * NT], pst[t], c0_col[b],
                        )
                (nc.sync if b == 0 else nc.scalar).dma_start(
                    out=out_d[b * P:(b + 1) * P, i * PIECE:(i + 1) * PIECE],
                    in_=ot,
                )


def build_program(enable_asserts=False):
    nc = bacc.Bacc(
        "TRN2",
        target_bir_lowering=False,
        debug=False,
        enable_asserts=enable_asserts,
        num_devices=8,
    )
    d_in = {
        "x0": nc.dram_tensor("x0", [P, NPIX], F16, kind="ExternalInput").ap(),
        "x1": nc.dram_tensor("x1", [P, NPIX], F16, kind="ExternalInput").ap(),
        "wcat": nc.dram_tensor("wcat", [C, 5 * C], F32,
                               kind="ExternalInput").ap(),
        "brows": nc.dram_tensor("brows", [5, C], F32,
                                kind="ExternalInput").ap(),
        "bcols": nc.dram_tensor("bcols", [C, 4], F32,
                                kind="ExternalInput").ap(),
        "ident": nc.dram_tensor("ident", [P, P], F32R,
                                kind="ExternalInput").ap(),
        "identh": nc.dram_tensor("identh", [P, P], F16,
                                 kind="ExternalInput").ap(),
    }
    d_out = {
        "out": nc.dram_tensor("out", [C, NPIX], F16,
                              kind="ExternalOutput").ap(),
    }
    with tile.TileContext(nc) as tc, ExitStack() as ctx:
        _emit(nc, tc, ctx, d_in, d_out)
    nc.compile()
    return nc


def make_in_maps(a, b, w1, b1, w2, b2, w3, b3, w4, b4):
    N = NPIX
    f = np.float32
    wcat = np.concatenate([w1.T, w1, w2.T, w3.T, w4],
                          axis=1).astype(f, copy=False)
    brows = np.stack([b1, N * b1, b2, b3, N * b3]).astype(f, copy=False)
    bcols = np.stack([b1, N * b1, b4, np.ones(C, f)], axis=1).astype(f)
    ident = np.eye(P, dtype=f)
    identh = np.eye(P, dtype=np.float16)
    B = a.shape[0]
    in_maps = []
    for i in range(B):
        in_maps.append({
            "x0": np.ascontiguousarray(a[i].reshape(P, N).astype(np.float16)),
            "x1": np.ascontiguousarray(b[i].reshape(P, N).astype(np.float16)),
            "wcat": wcat,
            "brows": brows,
            "bcols": bcols,
            "ident": ident,
            "identh": identh,
        })
    return in_maps


_CACHE = {}


def kernel(a, b, w1, b1, w2, b2, w3, b3, w4, b4, _trace=False):
    a = np.asarray(a, dtype=np.float32)
    b = np.asarray(b, dtype=np.float32)
    args = [np.asarray(t, dtype=np.float32)
            for t in (w1, b1, w2, b2, w3, b3, w4, b4)]
    if "nc" not in _CACHE:
        _CACHE["nc"] = build_program()
    nc = _CACHE["nc"]
    in_maps = make_in_maps(a, b, *args)
    res = run_bass_kernel_spmd(nc, in_maps, core_ids=list(range(8)),
                               trace=_trace)
    B, Ch, H, W = a.shape
    out = np.stack([r["out"].astype(np.float32).reshape(C, H, W)
                    for r in res.results])
    if _trace:
        _CACHE["last_results"] = res
    return out


# revision 10
# speedup vs baseline: 1.9730x; 1.3522x over previous
"""Trainium2 Bass kernel for nn_CLF_block (channel-attention block).

Reference computation (per batch item b, with x = concat([a,b], ch) in [256, N],
N = H*W = 16384):
    z  = w1 x + b1 1^T
    q  = w2 z + b2 1^T ;  k = w3 z + b3 1^T ;  v = w4 z + b4 1^T
    qk = q k^T ; attn = softmax(qk, -1) ; out = attn v

Algebraic restructuring with host-precomputed weight products
(A = w2 w1, B = w3 w1, E = w4 w1, rho2 = w2 b1 + b2, rho3 = w3 b1 + b3,
 w4b = w4 b1 + b4):
    Gx = x x^T ; sx = x 1                     (one fp16 pass over x)
    qk = A Gx B^T + (A sx) rho3^T + rho2 (B sx)^T + N rho2 rho3^T
    attn = softmax(qk)
    W  = attn E ; c0 = attn w4b
    out = W x + c0 1^T                        (second fp16 pass over x)

x is downloaded ONCE as fp16 in natural layout (8 MiB/core); the gram pass
consumes on-chip PE transposes of it (Gx symmetry halves the gram work).
The small algebra runs in f32r (single-pass PE).  Pass 2 is fp16 x fp16 and
the output is written fp16 (8 MiB/core), upcast on the host.

Sharding: data-parallel over batch, one batch item per NeuronCore (B=8, 8 cores).
"""

import sys

if "/opt/trn_rl_repo" not in sys.path:
    sys.path.insert(0, "/opt/trn_rl_repo")

from contextlib import ExitStack

import numpy as np

import concourse.bass as bass
import concourse.mybir as mybir
import concourse.tile as tile
from concourse import bacc
from concourse.bass_utils import run_bass_kernel_spmd

F32 = mybir.dt.float32
F32R = mybir.dt.float32r
F16 = mybir.dt.float16
P = 128           # partitions / channel block
C = 256           # channels
NPIX = 128 * 128  # spatial positions per batch item
NPIECE = 4        # x DMA pieces per channel half
PIECE = NPIX // NPIECE   # 4096 cols per piece
NCHUNK = NPIX // P       # 128 gram chunks
GRP = 8           # chunks per transpose group (one PSUM bank = 8 x [128,128] fp16)
NGRP = NCHUNK // GRP     # 16 groups
NT = 512          # matmul moving-operand width for pass 2


def _emit(nc, tc, ctx, d_in, d_out):
    """Emit the Tile program for one core (one batch item)."""
    wcat, ident, identh = d_in["wcat"], d_in["ident"], d_in["identh"]
    x0_d, x1_d = d_in["x0"], d_in["x1"]
    brows, bcols = d_in["brows"], d_in["bcols"]
    out_d = d_out["out"]

    const = ctx.enter_context(tc.tile_pool(name="const", bufs=1))
    xpool = ctx.enter_context(tc.tile_pool(name="xpool", bufs=1))

    # --- small constants (head of the scalar queue, ~0.4 MiB) ------------
    identh_sb = const.tile([P, P], F16, name="identh_sb", tag="identh_sb")
    nc.sync.dma_start(out=identh_sb, in_=identh[:, :])

    rows = []
    for r in range(3):
        rt = const.tile([1, C], F32R, name=f"brow{r}", tag=f"brow{r}")
        nc.scalar.dma_start(out=rt, in_=brows[r:r + 1, :])
        rows.append(rt)
    rho2_row, rho3_row, nrho3_row = rows

    bc_sb = []
    for k in range(2):
        bt = const.tile([P, 1], F32R, name=f"bcol{k}", tag=f"bcol{k}")
        nc.scalar.dma_start(out=bt, in_=bcols[k * P:(k + 1) * P, :])
        bc_sb.append(bt)
    w4b_col = [bc_sb[k][:, 0:1] for k in range(2)]

    ident_sb = const.tile([P, P], F32R, name="ident_sb", tag="ident_sb")
    nc.scalar.dma_start(out=ident_sb, in_=ident[:, :])

    # --- resident fp16 x in natural layout, pieces interleaved across ----
    # the two DMA queues so piece-pairs (x0[i], x1[i]) arrive together and
    # the big weight tensor does not delay the first pieces.
    xs = [[None] * NPIECE, [None] * NPIECE]
    for c in range(2):
        for i in range(NPIECE):
            xs[c][i] = xpool.tile([P, PIECE], F16, name=f"x{c}_{i}",
                                  tag=f"x{c}_{i}")
    for i in range(NPIECE):
        qa, qb = (nc.sync, nc.scalar) if i % 2 == 0 else (nc.scalar, nc.sync)
        qa.dma_start(out=xs[0][i], in_=x0_d[:, i * PIECE:(i + 1) * PIECE])
        qb.dma_start(out=xs[1][i], in_=x1_d[:, i * PIECE:(i + 1) * PIECE])

    # --- big weights last (needed only after the gram completes) ---------
    w_sb = []
    for k in range(2):
        wt = const.tile([P, 3 * C], F32R, name=f"w_sb{k}", tag=f"w_sb{k}")
        nc.scalar.dma_start(out=wt, in_=wcat[k * P:(k + 1) * P, :])
        w_sb.append(wt)
    At = [w_sb[k][:, 0 * C:1 * C] for k in range(2)]    # A^T  [c', o]
    Bt = [w_sb[k][:, 1 * C:2 * C] for k in range(2)]    # B^T  [c', o]
    E_ = [w_sb[k][:, 2 * C:3 * C] for k in range(2)]    # E    [j, c]

    # --- pass 1: Gx = x x^T via on-chip PE transposes --------------------
    # Per 128-col chunk: transpose both channel-half chunks into a PSUM
    # bank (8 chunks batched), copy to an ones-augmented fp16 staging tile,
    # then two accumulating gram matmuls.  Symmetry: only blocks (0,*) and
    # (1,1) are computed; block (1,0) = block (0,1)^T afterwards.
    stg = []
    for s in range(2):
        st = const.tile([P, GRP, C + 1], F16, name=f"stg{s}", tag=f"stg{s}")
        nc.vector.memset(st[:, :, C:C + 1], 1.0)
        stg.append(st)

    gx_sb = [
        const.tile([P, C + 1], F32R, name=f"gx_sb{b}", tag=f"gx_sb{b}")
        for b in range(2)
    ]

    CH_PP = PIECE // P  # chunks per piece
    with tc.tile_pool(name="gx_ps", bufs=1, space="PSUM") as gxp, \
         tc.tile_pool(name="tp_ps", bufs=1, space="PSUM") as tpp:
        shh0 = gxp.tile([P, C + 1], F32, name="shh0", tag="shh0")
        shh1 = gxp.tile([P, P + 1], F32, name="shh1", tag="shh1")

        def emit_transposes(grp):
            s = grp % 2
            tpA = tpp.tile([P, GRP, P], F16, name="tpA", tag=f"tpA{s}")
            tpB = tpp.tile([P, GRP, P], F16, name="tpB", tag=f"tpB{s}")
            for g in range(GRP):
                ch = grp * GRP + g
                i, lc = divmod(ch, CH_PP)
                nc.tensor.transpose(tpA[:, g, :],
                                    xs[0][i][:, lc * P:(lc + 1) * P],
                                    identh_sb)
                nc.tensor.transpose(tpB[:, g, :],
                                    xs[1][i][:, lc * P:(lc + 1) * P],
                                    identh_sb)
            nc.vector.tensor_copy(stg[s][:, :, 0:P], tpA)
            nc.scalar.activation(out=stg[s][:, :, P:C], in_=tpB,
                                 func=mybir.ActivationFunctionType.Identity,
                                 scale=1.0)

        def emit_grams(grp):
            s = grp % 2
            for g in range(GRP):
                ch = grp * GRP + g
                nc.tensor.matmul(shh0, stg[s][:, g, 0:P],
                                 stg[s][:, g, 0:C + 1],
                                 start=(ch == 0), stop=(ch == NCHUNK - 1))
                nc.tensor.matmul(shh1, stg[s][:, g, P:C],
                                 stg[s][:, g, P:C + 1],
                                 start=(ch == 0), stop=(ch == NCHUNK - 1))

        for grp in range(NGRP + 1):
            if grp < NGRP:
                emit_transposes(grp)
            if grp >= 1:
                emit_grams(grp - 1)

        # Assemble full Gx (with sx in col 256) from the symmetric pieces.
        nc.vector.tensor_copy(gx_sb[0], shh0)
        nc.vector.tensor_copy(gx_sb[1][:, P:C + 1], shh1)
        with tc.tile_pool(name="sym_ps", bufs=1, space="PSUM") as syp:
            tps = syp.tile([P, P], F32, name="tps", tag="tps")
            nc.tensor.transpose(tps, gx_sb[0][:, P:C].bitcast(F32),
                                ident_sb.bitcast(F32))
            nc.vector.tensor_copy(gx_sb[1][:, 0:P], tps)

    # Split the (large) diagonal out of Gx: products (Gx-D) A^T are much
    # smaller, so the PE's f32r per-product rounding no longer pollutes qk.
    # The diagonal term is applied exactly via per-partition multiplies.
    gxd = []
    for b in range(2):
        bs = slice(b * P, (b + 1) * P)
        dm = const.tile([P, P], F32, name=f"gxdm{b}", tag=f"gxdm{b}")
        nc.vector.tensor_mul(dm, gx_sb[b][:, bs].bitcast(F32), ident_sb.bitcast(F32))
        dcol = const.tile([P, 1], F32, name=f"gxd{b}", tag=f"gxd{b}")
        nc.vector.reduce_sum(out=dcol, in_=dm, axis=mybir.AxisListType.X)
        nc.vector.tensor_sub(gx_sb[b][:, bs], gx_sb[b][:, bs].bitcast(F32), dm)
        gxd.append(dcol)

    # --- tiny algebra: qk = A Gx B^T + rank-1 terms, softmax, W, c0 ------
    alg_sb = const  # persistent small tiles live in the const pool

    with tc.tile_pool(name="alg_ps", bufs=3, space="PSUM") as ap:
        # p_row = (A sx)^T, p3_row = (B sx)^T : lhsT = sx col (gx col 256)
        p_row = alg_sb.tile([1, C], F32R, name="p_row", tag="p_row")
        p3_row = alg_sb.tile([1, C], F32R, name="p3_row", tag="p3_row")
        for dst, wt in ((p_row, At), (p3_row, Bt)):
            vps = ap.tile([1, C], F32, name="vps", tag="algsmall", bufs=2)
            for k in range(2):
                nc.tensor.matmul(vps, gx_sb[k][:, C:C + 1],
                                 wt[k],
                                 start=(k == 0), stop=(k == 1))
            nc.vector.tensor_copy(dst, vps)

        # U' = (A (Gx-D))^T + D-correction : U'[j, o]
        u_sb = []
        for b in range(2):
            ups = ap.tile([P, C], F32, name="ups", tag="alg")
            for k in range(2):
                nc.tensor.matmul(ups,
                                 gx_sb[k][:, b * P:(b + 1) * P],
                                 At[k],
                                 start=(k == 0), stop=(k == 1))
            ud = alg_sb.tile([P, C], F32, name=f"u_d{b}", tag=f"u_d{b}")
            nc.vector.tensor_scalar_mul(ud, At[b].bitcast(F32), gxd[b])
            ut = alg_sb.tile([P, C], F32R, name=f"u_sb{b}", tag=f"u_sb{b}")
            nc.vector.tensor_add(ut, ups, ud)
            u_sb.append(ut)

        # qk = U'^T B^T + p rho3^T + rho2 p3^T + N rho2 rho3^T ; softmax
        attn_sb = []
        for b in range(2):
            qkps = ap.tile([P, C], F32, name="qkps", tag="alg")
            for k in range(2):
                nc.tensor.matmul(qkps,
                                 u_sb[k][:, b * P:(b + 1) * P],
                                 Bt[k], start=(k == 0),
                                 stop=False)
            nc.tensor.matmul(qkps, p_row[:, b * P:(b + 1) * P].bitcast(F32),
                             rho3_row.bitcast(F32), start=False, stop=False)
            nc.tensor.matmul(qkps, rho2_row[:, b * P:(b + 1) * P].bitcast(F32),
                             p3_row.bitcast(F32), start=False, stop=False)
            nc.tensor.matmul(qkps, rho2_row[:, b * P:(b + 1) * P].bitcast(F32),
                             nrho3_row.bitcast(F32), start=False, stop=True)

            negmax = alg_sb.tile([P, 1], F32, name=f"negmax{b}", tag=f"nm{b}")
            nc.vector.tensor_reduce(
                out=negmax, in_=qkps, op=mybir.AluOpType.max,
                axis=mybir.AxisListType.X, negate=True,
            )
            expq = alg_sb.tile([P, C], F32, name=f"expq{b}", tag=f"expq{b}")
            nc.scalar.activation(
                out=expq, in_=qkps, func=mybir.ActivationFunctionType.Exp,
                bias=negmax, scale=1.0,
            )
            denom = alg_sb.tile([P, 1], F32, name=f"denom{b}", tag=f"dn{b}")
            nc.vector.reduce_sum(out=denom, in_=expq,
                                 axis=mybir.AxisListType.X)
            rden = alg_sb.tile([P, 1], F32, name=f"rden{b}", tag=f"rd{b}")
            nc.vector.reciprocal(rden, denom)
            at = alg_sb.tile([P, C], F32, name=f"attn{b}", tag=f"attn{b}")
            nc.vector.tensor_scalar_mul(at, expq, rden)
            attn_sb.append(at)

        # attn^T (4 PE transposes)
        attnT_sb = [
            alg_sb.tile([P, C], F32R, name=f"attnT{j}", tag=f"attnT{j}")
            for j in range(2)
        ]
        for b in range(2):
            for j in range(2):
                tps = ap.tile([P, P], F32, name="tps", tag="algtp", bufs=2)
                nc.tensor.transpose(tps,
                                    attn_sb[b][:, j * P:(j + 1) * P],
                                    ident_sb.bitcast(F32))
                nc.vector.tensor_copy(attnT_sb[j][:, b * P:(b + 1) * P], tps)

        # W^T = E-as-lhsT @ attn^T  (stored fp16 for the fp16 pass 2)
        wt_sb = []
        for b in range(2):
            wps = ap.tile([P, C], F32, name="wps", tag="alg")
            for k in range(2):
                nc.tensor.matmul(wps, E_[k][:, b * P:(b + 1) * P],
                                 attnT_sb[k],
                                 start=(k == 0), stop=(k == 1))
            wt_ = alg_sb.tile([P, C], F16, name=f"wt_sb{b}", tag=f"wt_sb{b}")
            nc.vector.tensor_copy(wt_, wps)
            wt_sb.append(wt_)

        # c0_col = attn w4b (per block)
        c0_col = []
        for b in range(2):
            cps = ap.tile([P, 1], F32, name="cps", tag="algsmall", bufs=2)
            for k in range(2):
                nc.tensor.matmul(cps,
                                 attnT_sb[k][:, b * P:(b + 1) * P].bitcast(F32),
                                 w4b_col[k].bitcast(F32), start=(k == 0),
                                 stop=(k == 1))
            ct = alg_sb.tile([P, 1], F32, name=f"c0_col{b}", tag=f"c0_col{b}")
            nc.vector.tensor_copy(ct, cps)
            c0_col.append(ct)

    # --- pass 2: out = W x + c0 1^T (all fp16 operands, fp16 output) -----
    # PSUM double-tiles [128, 2, 512] (2 banks each, 4 bufs = all 8 banks);
    # one batched drain per unit, alternating scalar/vector engines.
    with tc.tile_pool(name="o_ps", bufs=4, space="PSUM") as ops, \
         tc.tile_pool(name="o_sb", bufs=4) as osb:
        nsub = 2
        SUBP = nsub * NT  # 1024 cols per unit
        units = []
        for i in range(NPIECE):
            for b in range(2):
                for u in range(PIECE // SUBP):
                    units.append((i, b, u))
        for n, (i, b, u) in enumerate(units):
            ot = osb.tile([P, nsub, NT], F16, name="ot", tag="ot")
            pst = ops.tile([P, nsub, NT], F32, name="pst", tag="pst")
            for k in range(2):
                for t in range(nsub):
                    nc.tensor.matmul(
                        pst[:, t, :],
                        wt_sb[k][:, b * P:(b + 1) * P],
                        xs[k][i][:, u * SUBP + t * NT:u * SUBP + (t + 1) * NT],
                        start=(k == 0),
                        stop=(k == 1),
                    )
            if n % 2 == 0:
                nc.scalar.activation(
                    out=ot, in_=pst,
                    func=mybir.ActivationFunctionType.Identity,
                    bias=c0_col[b], scale=1.0,
                )
            else:
                nc.vector.tensor_scalar_add(ot, pst, c0_col[b])
            (nc.sync if n % 2 == 0 else nc.scalar).dma_start(
                out=out_d[b * P:(b + 1) * P,
                          i * PIECE + u * SUBP:i * PIECE + (u + 1) * SUBP],
                in_=ot,
            )


def build_program(enable_asserts=False):
    nc = bacc.Bacc(
        "TRN2",
        target_bir_lowering=False,
        debug=False,
        enable_asserts=enable_asserts,
        num_devices=8,
    )
    d_in = {
        "x0": nc.dram_tensor("x0", [P, NPIX], F16, kind="ExternalInput").ap(),
        "x1": nc.dram_tensor("x1", [P, NPIX], F16, kind="ExternalInput").ap(),
        "wcat": nc.dram_tensor("wcat", [C, 3 * C], F32R,
                               kind="ExternalInput").ap(),
        "brows": nc.dram_tensor("brows", [3, C], F32R,
                                kind="ExternalInput").ap(),
        "bcols": nc.dram_tensor("bcols", [C, 1], F32R,
                                kind="ExternalInput").ap(),
        "ident": nc.dram_tensor("ident", [P, P], F32R,
                                kind="ExternalInput").ap(),
        "identh": nc.dram_tensor("identh", [P, P], F16,
                                 kind="ExternalInput").ap(),
    }
    d_out = {
        "out": nc.dram_tensor("out", [C, NPIX], F16,
                              kind="ExternalOutput").ap(),
    }
    with tile.TileContext(nc) as tc, ExitStack() as ctx:
        _emit(nc, tc, ctx, d_in, d_out)
    nc.compile()
    return nc


def make_in_maps(a, b, w1, b1, w2, b2, w3, b3, w4, b4):
    N = NPIX
    f = np.float32
    A = (w2 @ w1).astype(f)
    B = (w3 @ w1).astype(f)
    E = (w4 @ w1).astype(f)
    rho2 = (w2 @ b1 + b2).astype(f)
    rho3 = (w3 @ b1 + b3).astype(f)
    w4b = (w4 @ b1 + b4).astype(f)
    wcat = _round_f32r(np.ascontiguousarray(
        np.concatenate([A.T, B.T, E], axis=1).astype(f, copy=False)))
    brows = _round_f32r(np.stack([rho2, rho3, N * rho3]).astype(f))
    bcols = _round_f32r(w4b[:, None].astype(f))
    ident = np.eye(P, dtype=f)
    identh = np.eye(P, dtype=np.float16)
    Bsz = a.shape[0]
    in_maps = []
    for i in range(Bsz):
        in_maps.append({
            "x0": np.ascontiguousarray(a[i].reshape(P, N).astype(np.float16)),
            "x1": np.ascontiguousarray(b[i].reshape(P, N).astype(np.float16)),
            "wcat": wcat,
            "brows": brows,
            "bcols": bcols,
            "ident": ident,
            "identh": identh,
        })
    return in_maps


def _round_f32r(x):
    """Round fp32 to the FP32R-representable set (hi-bf16 + lo-bf16)."""
    import ml_dtypes

    x = np.asarray(x, np.float32)
    hi = x.astype(ml_dtypes.bfloat16).astype(np.float32)
    lo = (x - hi).astype(ml_dtypes.bfloat16).astype(np.float32)
    return np.ascontiguousarray(hi + lo)


_CACHE = {}


def kernel(a, b, w1, b1, w2, b2, w3, b3, w4, b4, _trace=False):
    a = np.asarray(a, dtype=np.float32)
    b = np.asarray(b, dtype=np.float32)
    args = [np.asarray(t, dtype=np.float32)
            for t in (w1, b1, w2, b2, w3, b3, w4, b4)]
    if "nc" not in _CACHE:
        _CACHE["nc"] = build_program()
    nc = _CACHE["nc"]
    in_maps = make_in_maps(a, b, *args)
    res = run_bass_kernel_spmd(nc, in_maps, core_ids=list(range(8)),
                               trace=_trace)
    B, Ch, H, W = a.shape
    out = np.stack([r["out"].astype(np.float32).reshape(C, H, W)
                    for r in res.results])
    if _trace:
        _CACHE["last_results"] = res
    return out


# revision 11
# speedup vs baseline: 2.1658x; 1.0977x over previous
"""Trainium2 Bass kernel for nn_CLF_block (channel-attention block).

Reference computation (per batch item b, with x = concat([a,b], ch) in [256, N],
N = H*W = 16384):
    z  = w1 x + b1 1^T
    q  = w2 z + b2 1^T ;  k = w3 z + b3 1^T ;  v = w4 z + b4 1^T
    qk = q k^T ; attn = softmax(qk, -1) ; out = attn v

Algebraic restructuring with host-precomputed weight products
(A = w2 w1, B = w3 w1, E = w4 w1, rho2 = w2 b1 + b2, rho3 = w3 b1 + b3,
 w4b = w4 b1 + b4):
    Gx = x x^T ; sx = x 1                     (one fp16 pass over x)
    qk = A Gx B^T + (A sx) rho3^T + rho2 (B sx)^T + N rho2 rho3^T
    attn = softmax(qk)
    W  = attn E ; c0 = attn w4b
    out = W x + c0 1^T                        (second fp16 pass over x)

x is downloaded ONCE as fp16 in natural layout (8 MiB/core); the gram pass
consumes on-chip PE transposes of it (Gx symmetry halves the gram work).
The small algebra runs in f32r (single-pass PE).  Pass 2 is fp16 x fp16 and
the output is written fp16 (8 MiB/core), upcast on the host.

Sharding: data-parallel over batch, one batch item per NeuronCore (B=8, 8 cores).
"""

import sys

if "/opt/trn_rl_repo" not in sys.path:
    sys.path.insert(0, "/opt/trn_rl_repo")

from contextlib import ExitStack

import numpy as np

import concourse.bass as bass
import concourse.mybir as mybir
import concourse.tile as tile
from concourse import bacc
from concourse.bass_utils import run_bass_kernel_spmd

F32 = mybir.dt.float32
F32R = mybir.dt.float32r
F16 = mybir.dt.float16
P = 128           # partitions / channel block
C = 256           # channels
NPIX = 128 * 128  # spatial positions per batch item
NPIECE = 4        # x DMA pieces per channel half
PIECE = NPIX // NPIECE   # 4096 cols per piece
NCHUNK = NPIX // P       # 128 gram chunks
GRP = 8           # chunks per transpose group (one PSUM bank = 8 x [128,128] fp16)
NGRP = NCHUNK // GRP     # 16 groups
NT = 512          # matmul moving-operand width for pass 2


def _emit(nc, tc, ctx, d_in, d_out):
    """Emit the Tile program for one core (one batch item)."""
    wcat, ident, identh = d_in["wcat"], d_in["ident"], d_in["identh"]
    x0_d, x1_d = d_in["x0"], d_in["x1"]
    brows, bcols = d_in["brows"], d_in["bcols"]
    out_d = d_out["out"]

    const = ctx.enter_context(tc.tile_pool(name="const", bufs=1))
    xpool = ctx.enter_context(tc.tile_pool(name="xpool", bufs=1))

    # --- small constants (head of the scalar queue, ~0.4 MiB) ------------
    identh_sb = const.tile([P, P], F16, name="identh_sb", tag="identh_sb")
    nc.sync.dma_start(out=identh_sb, in_=identh[:, :])

    rows = []
    for r in range(3):
        rt = const.tile([1, C], F32R, name=f"brow{r}", tag=f"brow{r}")
        nc.scalar.dma_start(out=rt, in_=brows[r:r + 1, :])
        rows.append(rt)
    rho2_row, rho3_row, nrho3_row = rows

    bc_sb = []
    for k in range(2):
        bt = const.tile([P, 1], F32R, name=f"bcol{k}", tag=f"bcol{k}")
        nc.scalar.dma_start(out=bt, in_=bcols[k * P:(k + 1) * P, :])
        bc_sb.append(bt)
    w4b_col = [bc_sb[k][:, 0:1] for k in range(2)]

    ident_sb = const.tile([P, P], F32R, name="ident_sb", tag="ident_sb")
    nc.scalar.dma_start(out=ident_sb, in_=ident[:, :])

    # --- fp16 E for the W = attn E product (small, scalar queue) ---------
    eh = d_in["eh"]
    eh_sb = []
    for k in range(2):
        et = const.tile([P, C], F16, name=f"eh_sb{k}", tag=f"eh_sb{k}")
        nc.scalar.dma_start(out=et, in_=eh[k * P:(k + 1) * P, :])
        eh_sb.append(et)

    # --- resident fp16 x in natural layout.  All x pieces go on the sync
    # queue (scalar must stay free for phase-1 staging copies); the first
    # piece is split into 1024-col sub-DMAs so the PE can start early.
    xs = [[None] * NPIECE, [None] * NPIECE]
    for c in range(2):
        for i in range(NPIECE):
            xs[c][i] = xpool.tile([P, PIECE], F16, name=f"x{c}_{i}",
                                  tag=f"x{c}_{i}")
    SUB = 1024
    for s in range(PIECE // SUB):
        for c, src_d in ((0, x0_d), (1, x1_d)):
            nc.sync.dma_start(out=xs[c][0][:, s * SUB:(s + 1) * SUB],
                              in_=src_d[:, s * SUB:(s + 1) * SUB])
    for i in range(1, NPIECE):
        for c, src_d in ((0, x0_d), (1, x1_d)):
            nc.sync.dma_start(out=xs[c][i],
                              in_=src_d[:, i * PIECE:(i + 1) * PIECE])

    # --- big weights last on sync (needed only after the gram) -----------
    w_sb = []
    for k in range(2):
        wt = const.tile([P, 2 * C], F32R, name=f"w_sb{k}", tag=f"w_sb{k}")
        nc.sync.dma_start(out=wt, in_=wcat[k * P:(k + 1) * P, :])
        w_sb.append(wt)
    At = [w_sb[k][:, 0 * C:1 * C] for k in range(2)]    # A^T  [c', o]
    Bt = [w_sb[k][:, 1 * C:2 * C] for k in range(2)]    # B^T  [c', o]

    # --- pass 1: Gx = x x^T via on-chip PE transposes --------------------
    # Per 128-col chunk: transpose both channel-half chunks into a PSUM
    # bank (8 chunks batched), copy to an ones-augmented fp16 staging tile,
    # then two accumulating gram matmuls.  Symmetry: only blocks (0,*) and
    # (1,1) are computed; block (1,0) = block (0,1)^T afterwards.
    stg = []
    for s in range(2):
        st = const.tile([P, GRP, C + 1], F16, name=f"stg{s}", tag=f"stg{s}")
        nc.vector.memset(st[:, :, C:C + 1], 1.0)
        stg.append(st)

    gx_sb = [
        const.tile([P, C + 1], F32R, name=f"gx_sb{b}", tag=f"gx_sb{b}")
        for b in range(2)
    ]

    CH_PP = PIECE // P  # chunks per piece
    with tc.tile_pool(name="gx_ps", bufs=1, space="PSUM") as gxp, \
         tc.tile_pool(name="tp_ps", bufs=1, space="PSUM") as tpp:
        shh0 = gxp.tile([P, C + 1], F32, name="shh0", tag="shh0")
        shh1 = gxp.tile([P, P + 1], F32, name="shh1", tag="shh1")

        def emit_transposes(grp):
            s = grp % 2
            tpA = tpp.tile([P, GRP, P], F16, name="tpA", tag=f"tpA{s}")
            tpB = tpp.tile([P, GRP, P], F16, name="tpB", tag=f"tpB{s}")
            for g in range(GRP):
                ch = grp * GRP + g
                i, lc = divmod(ch, CH_PP)
                nc.tensor.transpose(tpA[:, g, :],
                                    xs[0][i][:, lc * P:(lc + 1) * P],
                                    identh_sb)
                nc.tensor.transpose(tpB[:, g, :],
                                    xs[1][i][:, lc * P:(lc + 1) * P],
                                    identh_sb)
            nc.vector.tensor_copy(stg[s][:, :, 0:P], tpA)
            nc.scalar.activation(out=stg[s][:, :, P:C], in_=tpB,
                                 func=mybir.ActivationFunctionType.Identity,
                                 scale=1.0)

        def emit_grams(grp):
            s = grp % 2
            for g in range(GRP):
                ch = grp * GRP + g
                nc.tensor.matmul(shh0, stg[s][:, g, 0:P],
                                 stg[s][:, g, 0:C + 1],
                                 start=(ch == 0), stop=(ch == NCHUNK - 1))
                nc.tensor.matmul(shh1, stg[s][:, g, P:C],
                                 stg[s][:, g, P:C + 1],
                                 start=(ch == 0), stop=(ch == NCHUNK - 1))

        for grp in range(NGRP + 1):
            if grp < NGRP:
                emit_transposes(grp)
            if grp >= 1:
                emit_grams(grp - 1)

        # Assemble full Gx (with sx in col 256) from the symmetric pieces.
        nc.vector.tensor_copy(gx_sb[0], shh0)
        nc.vector.tensor_copy(gx_sb[1][:, P:C + 1], shh1)
        with tc.tile_pool(name="sym_ps", bufs=1, space="PSUM") as syp:
            tps = syp.tile([P, P], F32, name="tps", tag="tps")
            nc.tensor.transpose(tps, gx_sb[0][:, P:C].bitcast(F32),
                                ident_sb.bitcast(F32))
            nc.vector.tensor_copy(gx_sb[1][:, 0:P], tps)

    # Split the (large) diagonal out of Gx: products (Gx-D) A^T are much
    # smaller, so the PE's f32r per-product rounding no longer pollutes qk.
    # The diagonal term is applied exactly via per-partition multiplies.
    gxd = []
    for b in range(2):
        bs = slice(b * P, (b + 1) * P)
        dm = const.tile([P, P], F32, name=f"gxdm{b}", tag=f"gxdm{b}")
        nc.vector.tensor_mul(dm, gx_sb[b][:, bs].bitcast(F32), ident_sb.bitcast(F32))
        dcol = const.tile([P, 1], F32, name=f"gxd{b}", tag=f"gxd{b}")
        nc.vector.reduce_sum(out=dcol, in_=dm, axis=mybir.AxisListType.X)
        nc.vector.tensor_sub(gx_sb[b][:, bs], gx_sb[b][:, bs].bitcast(F32), dm)
        gxd.append(dcol)

    # --- tiny algebra: qk = A Gx B^T + rank-1 terms, softmax, W, c0 ------
    alg_sb = const  # persistent small tiles live in the const pool

    with tc.tile_pool(name="alg_ps", bufs=3, space="PSUM") as ap:
        # p_row = (A sx)^T, p3_row = (B sx)^T : lhsT = sx col (gx col 256)
        p_row = alg_sb.tile([1, C], F32R, name="p_row", tag="p_row")
        p3_row = alg_sb.tile([1, C], F32R, name="p3_row", tag="p3_row")
        for dst, wt in ((p_row, At), (p3_row, Bt)):
            vps = ap.tile([1, C], F32, name="vps", tag="algsmall", bufs=2)
            for k in range(2):
                nc.tensor.matmul(vps, gx_sb[k][:, C:C + 1],
                                 wt[k],
                                 start=(k == 0), stop=(k == 1))
            nc.vector.tensor_copy(dst, vps)

        # U' = (A (Gx-D))^T + D-correction : U'[j, o]
        u_sb = []
        for b in range(2):
            ups = ap.tile([P, C], F32, name="ups", tag="alg")
            for k in range(2):
                nc.tensor.matmul(ups,
                                 gx_sb[k][:, b * P:(b + 1) * P].bitcast(F32),
                                 At[k].bitcast(F32),
                                 start=(k == 0), stop=(k == 1))
            ud = alg_sb.tile([P, C], F32, name=f"u_d{b}", tag=f"u_d{b}")
            nc.vector.tensor_scalar_mul(ud, At[b].bitcast(F32), gxd[b])
            ut = alg_sb.tile([P, C], F32, name=f"u_sb{b}", tag=f"u_sb{b}")
            nc.vector.tensor_add(ut, ups, ud)
            u_sb.append(ut)

        # qk = U'^T B^T + p rho3^T + rho2 p3^T + N rho2 rho3^T ; softmax
        attn_sb = []
        for b in range(2):
            qkps = ap.tile([P, C], F32, name="qkps", tag="alg")
            for k in range(2):
                nc.tensor.matmul(qkps,
                                 u_sb[k][:, b * P:(b + 1) * P],
                                 Bt[k].bitcast(F32), start=(k == 0),
                                 stop=False)
            nc.tensor.matmul(qkps, p_row[:, b * P:(b + 1) * P].bitcast(F32),
                             rho3_row.bitcast(F32), start=False, stop=False)
            nc.tensor.matmul(qkps, rho2_row[:, b * P:(b + 1) * P].bitcast(F32),
                             p3_row.bitcast(F32), start=False, stop=False)
            nc.tensor.matmul(qkps, rho2_row[:, b * P:(b + 1) * P].bitcast(F32),
                             nrho3_row.bitcast(F32), start=False, stop=True)

            negmax = alg_sb.tile([P, 1], F32, name=f"negmax{b}", tag=f"nm{b}")
            nc.vector.tensor_reduce(
                out=negmax, in_=qkps, op=mybir.AluOpType.max,
                axis=mybir.AxisListType.X, negate=True,
            )
            expq = alg_sb.tile([P, C], F32, name=f"expq{b}", tag=f"expq{b}")
            nc.scalar.activation(
                out=expq, in_=qkps, func=mybir.ActivationFunctionType.Exp,
                bias=negmax, scale=1.0,
            )
            denom = alg_sb.tile([P, 1], F32, name=f"denom{b}", tag=f"dn{b}")
            nc.vector.reduce_sum(out=denom, in_=expq,
                                 axis=mybir.AxisListType.X)
            rden = alg_sb.tile([P, 1], F32, name=f"rden{b}", tag=f"rd{b}")
            nc.vector.reciprocal(rden, denom)
            at = alg_sb.tile([P, C], F32, name=f"attn{b}", tag=f"attn{b}")
            nc.vector.tensor_scalar_mul(at, expq, rden)
            attn_sb.append(at)

        # attn^T (4 PE transposes); fp16 copy for the W product and an
        # f32 copy for the fp32 c0 matmul
        attnT_sb = [
            alg_sb.tile([P, C], F32, name=f"attnT{j}", tag=f"attnT{j}")
            for j in range(2)
        ]
        attnT16 = [
            alg_sb.tile([P, C], F16, name=f"attnT16_{j}", tag=f"attnT16_{j}")
            for j in range(2)
        ]
        for b in range(2):
            for j in range(2):
                tps = ap.tile([P, P], F32, name="tps", tag="algtp", bufs=2)
                nc.tensor.transpose(tps,
                                    attn_sb[b][:, j * P:(j + 1) * P],
                                    ident_sb.bitcast(F32))
                nc.vector.tensor_copy(attnT_sb[j][:, b * P:(b + 1) * P], tps)
                nc.vector.tensor_copy(attnT16[j][:, b * P:(b + 1) * P], tps)

        # W^T = E-as-lhsT @ attn^T, all fp16 (stored fp16 for pass 2)
        wt_sb = []
        for b in range(2):
            wps = ap.tile([P, C], F32, name="wps", tag="alg")
            for k in range(2):
                nc.tensor.matmul(wps, eh_sb[k][:, b * P:(b + 1) * P],
                                 attnT16[k],
                                 start=(k == 0), stop=(k == 1))
            wt_ = alg_sb.tile([P, C], F16, name=f"wt_sb{b}", tag=f"wt_sb{b}")
            nc.vector.tensor_copy(wt_, wps)
            wt_sb.append(wt_)

        # c0_col = attn w4b (per block)
        c0_col = []
        for b in range(2):
            cps = ap.tile([P, 1], F32, name="cps", tag="algsmall", bufs=2)
            for k in range(2):
                nc.tensor.matmul(cps,
                                 attnT_sb[k][:, b * P:(b + 1) * P],
                                 w4b_col[k].bitcast(F32), start=(k == 0),
                                 stop=(k == 1))
            ct = alg_sb.tile([P, 1], F32, name=f"c0_col{b}", tag=f"c0_col{b}")
            nc.vector.tensor_copy(ct, cps)
            c0_col.append(ct)

    # --- pass 2: out = W x + c0 1^T (all fp16 operands, fp16 output) -----
    # PSUM double-tiles [128, 2, 512] (2 banks each, 4 bufs = all 8 banks);
    # one batched drain per unit, alternating scalar/vector engines.
    with tc.tile_pool(name="o_ps", bufs=4, space="PSUM") as ops, \
         tc.tile_pool(name="o_sb", bufs=4) as osb:
        nsub = 2
        SUBP = nsub * NT  # 1024 cols per unit
        units = []
        for i in range(NPIECE):
            for b in range(2):
                for u in range(PIECE // SUBP):
                    units.append((i, b, u))
        for n, (i, b, u) in enumerate(units):
            ot = osb.tile([P, nsub, NT], F16, name="ot", tag="ot")
            pst = ops.tile([P, nsub, NT], F32, name="pst", tag="pst")
            for k in range(2):
                for t in range(nsub):
                    nc.tensor.matmul(
                        pst[:, t, :],
                        wt_sb[k][:, b * P:(b + 1) * P],
                        xs[k][i][:, u * SUBP + t * NT:u * SUBP + (t + 1) * NT],
                        start=(k == 0),
                        stop=(k == 1),
                    )
            if n % 2 == 0:
                nc.scalar.activation(
                    out=ot, in_=pst,
                    func=mybir.ActivationFunctionType.Identity,
                    bias=c0_col[b], scale=1.0,
                )
            else:
                nc.vector.tensor_scalar_add(ot, pst, c0_col[b])
            nc.sync.dma_start(
                out=out_d[b * P:(b + 1) * P,
                          i * PIECE + u * SUBP:i * PIECE + (u + 1) * SUBP],
                in_=ot,
            )


def build_program(enable_asserts=False):
    nc = bacc.Bacc(
        "TRN2",
        target_bir_lowering=False,
        debug=False,
        enable_asserts=enable_asserts,
        num_devices=8,
    )
    d_in = {
        "x0": nc.dram_tensor("x0", [P, NPIX], F16, kind="ExternalInput").ap(),
        "x1": nc.dram_tensor("x1", [P, NPIX], F16, kind="ExternalInput").ap(),
        "wcat": nc.dram_tensor("wcat", [C, 2 * C], F32R,
                               kind="ExternalInput").ap(),
        "brows": nc.dram_tensor("brows", [3, C], F32R,
                                kind="ExternalInput").ap(),
        "bcols": nc.dram_tensor("bcols", [C, 1], F32R,
                                kind="ExternalInput").ap(),
        "ident": nc.dram_tensor("ident", [P, P], F32R,
                                kind="ExternalInput").ap(),
        "identh": nc.dram_tensor("identh", [P, P], F16,
                                 kind="ExternalInput").ap(),
        "eh": nc.dram_tensor("eh", [C, C], F16, kind="ExternalInput").ap(),
    }
    d_out = {
        "out": nc.dram_tensor("out", [C, NPIX], F16,
                              kind="ExternalOutput").ap(),
    }
    with tile.TileContext(nc) as tc, ExitStack() as ctx:
        _emit(nc, tc, ctx, d_in, d_out)
    nc.compile()
    return nc


def make_in_maps(a, b, w1, b1, w2, b2, w3, b3, w4, b4):
    N = NPIX
    f = np.float32
    A = (w2 @ w1).astype(f)
    B = (w3 @ w1).astype(f)
    E = (w4 @ w1).astype(f)
    rho2 = (w2 @ b1 + b2).astype(f)
    rho3 = (w3 @ b1 + b3).astype(f)
    w4b = (w4 @ b1 + b4).astype(f)
    wcat = _round_f32r(np.ascontiguousarray(
        np.concatenate([A.T, B.T], axis=1).astype(f, copy=False)))
    eh16 = np.ascontiguousarray(E.astype(np.float16))
    brows = _round_f32r(np.stack([rho2, rho3, N * rho3]).astype(f))
    bcols = _round_f32r(w4b[:, None].astype(f))
    ident = np.eye(P, dtype=f)
    identh = np.eye(P, dtype=np.float16)
    Bsz = a.shape[0]
    in_maps = []
    for i in range(Bsz):
        in_maps.append({
            "x0": np.ascontiguousarray(a[i].reshape(P, N).astype(np.float16)),
            "x1": np.ascontiguousarray(b[i].reshape(P, N).astype(np.float16)),
            "wcat": wcat,
            "brows": brows,
            "bcols": bcols,
            "ident": ident,
            "identh": identh,
            "eh": eh16,
        })
    return in_maps


def _round_f32r(x):
    """Round fp32 to the FP32R-representable set (hi-bf16 + lo-bf16)."""
    import ml_dtypes

    x = np.asarray(x, np.float32)
    hi = x.astype(ml_dtypes.bfloat16).astype(np.float32)
    lo = (x - hi).astype(ml_dtypes.bfloat16).astype(np.float32)
    return np.ascontiguousarray(hi + lo)


_CACHE = {}


def kernel(a, b, w1, b1, w2, b2, w3, b3, w4, b4, _trace=False):
    a = np.asarray(a, dtype=np.float32)
    b = np.asarray(b, dtype=np.float32)
    args = [np.asarray(t, dtype=np.float32)
            for t in (w1, b1, w2, b2, w3, b3, w4, b4)]
    if "nc" not in _CACHE:
        _CACHE["nc"] = build_program()
    nc = _CACHE["nc"]
    in_maps = make_in_maps(a, b, *args)
    res = run_bass_kernel_spmd(nc, in_maps, core_ids=list(range(8)),
                               trace=_trace)
    B, Ch, H, W = a.shape
    out = np.stack([r["out"].astype(np.float32).reshape(C, H, W)
                    for r in res.results])
    if _trace:
        _CACHE["last_results"] = res
    return out
